# revision 1
# baseline (speedup 1.0000x reference)
"""CMAttention Trainium2 Bass kernel.

Reference computation (b=2, n=2048, dim=512, H=8 heads, dh=64, M=3 memory tokens):
    q = x @ wq;  k, v = split(x @ wkv);  per-head attention with 3 extra
    memory k/v tokens appended;  out = softmax(q k^T / 8) v;  y = out @ wo + bo.

Sharding: 16 (batch, head) pairs over 8 cores -> each core owns one batch and
two adjacent heads.  Per core everything is local; the out-projection is
row-sharded (per-head) and partial outputs are summed on the host (the
all-reduce of the sharding hint, done at gather time).

Device-side layout (per core, two heads "stacked" on partitions 0-63 / 64-127):
    xT   [4][128, 2048]   x[b]^T, contraction dim c on partitions (host pre-transposed)
    qT   [128, 2048]      q^T = wq_s^T-chunks @ xT    (d_global on partitions)
    kT   [128, 2052]      k^T * 1/8 (scale folded into wk on host) ++ memory keys
    v    via PE transpose -> v_aug[h] [128, 17*65]: per j-tile [128, 64+1(ones)]
    scores^T s[j, i] = kT_h^T-slice.T @ qT_h  -> PSUM [128(j), 1024(i)]
    exp on ScalarE PSUM->SBUF
    av:  out_h^T[65, i] += v_aug_jt.T @ exp_jt   (row 64 = softmax denominator)
    out-projection per head + per-partition reciprocal scaling, host sums partials.
"""

import sys

for _p in ("/opt/trn_rl_repo", "/root/.axon_site/_ro/trn_rl_repo"):
    if _p not in sys.path:
        sys.path.insert(0, _p)

import ml_dtypes
import numpy as np

import concourse.bacc as bacc
import concourse.mybir as mybir
import concourse.tile as tile
from concourse import bass_utils
from concourse.masks import make_identity

F32 = mybir.dt.float32
FR = mybir.dt.float32r  # fp32 bits, single-pass PE matmul (1 cycle/row for N>=256)
BF = mybir.dt.bfloat16
AF = mybir.ActivationFunctionType
ALU = mybir.AluOpType

H, DH, M = 8, 64, 3
DIM = 512
INNER = H * DH
NSEQ = 2048
B = 2
N_CORES = 8
SCALE = DH ** -0.5
SQRT_M = float(np.sqrt(M))

_CACHE = {}


def _emit(nc, tc, n):
    """Emit the per-core program. n = sequence length (queries)."""
    n_it = n // 128          # i-tiles of 128 queries
    n_ic = n // 512          # 512-query column chunks
    n_ih = n // 1024         # 1024-query halves for the attention loop
    n_jt = n // 128 + 1      # j-tiles: n/128 full + 1 memory tile (3 rows)
    VA = 65                  # v_aug cols per j-tile: 64 dims + ones column

    ap_xt = nc.dram_tensor("xt", [4, 128, n], FR, kind="ExternalInput").ap()
    ap_wq = nc.dram_tensor("wq_s", [4, 128, 128], FR, kind="ExternalInput").ap()
    ap_wk = nc.dram_tensor("wk_s", [4, 128, 128], FR, kind="ExternalInput").ap()
    ap_wv = nc.dram_tensor("wv_s", [4, 128, 128], FR, kind="ExternalInput").ap()
    ap_wo = nc.dram_tensor("wo_s", [128, DIM], BF, kind="ExternalInput").ap()
    ap_mkT = nc.dram_tensor("mkT_s", [128, M], BF, kind="ExternalInput").ap()
    ap_mv = nc.dram_tensor("mv_s", [M, 128], BF, kind="ExternalInput").ap()
    ap_out = nc.dram_tensor("out", [n_it, 128, DIM], F32, kind="ExternalOutput").ap()

    with (
        tc.tile_pool(name="persist", bufs=1) as per,
        tc.tile_pool(name="dram", bufs=1, space="DRAM") as dpool,
    ):
        xt = [per.tile([128, n], FR, tag=f"xt{c}", name=f"xt{c}") for c in range(4)]
        wq_sb = [per.tile([128, 128], FR, tag=f"wq{c}", name=f"wq{c}") for c in range(4)]
        wk_sb = [per.tile([128, 128], FR, tag=f"wk{c}", name=f"wk{c}") for c in range(4)]
        wv_sb = [per.tile([128, 128], FR, tag=f"wv{c}", name=f"wv{c}") for c in range(4)]
        wo_sb = per.tile([128, DIM], BF, tag="wo", name="wo")
        qT = per.tile([128, n], BF, tag="qT", name="qT")
        kT = per.tile([128, n + 128], BF, tag="kT", name="kT")
        vT = per.tile([128, n], F32, tag="vT", name="vT")
        v_aug = [per.tile([128, n_jt * VA], BF, tag=f"vaug{h}", name=f"vaug{h}") for h in range(2)]
        oT = per.tile([128, n], BF, tag="oT", name="oT")
        rec_col = [per.tile([128, n_it], F32, tag=f"rec{h}", name=f"rec{h}") for h in range(2)]
        ident = per.tile([128, 128], F32, tag="ident", name="ident")

        nc.sync.dma_start(out=wo_sb, in_=ap_wo)
        nc.vector.memset(kT[:, n : n + 128], 0.0)
        nc.sync.dma_start(out=kT[:, n : n + M], in_=ap_mkT)
        make_identity(nc, ident[:])
        for h in range(2):
            nc.vector.memset(v_aug[h][:], 1.0)
            mb = (n_jt - 1) * VA
            nc.vector.memset(v_aug[h][:, mb : mb + VA], 0.0)
            nc.vector.memset(v_aug[h][0:M, mb + 64 : mb + VA], 1.0)
            nc.sync.dma_start(
                out=v_aug[h][0:M, mb : mb + 64],
                in_=ap_mv[:, h * 64 : (h + 1) * 64],
            )

        # ---- minimal pre-attention projections: kT (ic-major so early j-tiles
        # land first) and qT chunk 0. vT + transposes + remaining qT chunks are
        # deferred into the attention window (PE slack under the exp stream).
        with tc.tile_pool(name="proj_ps", bufs=6, space="PSUM") as proj_ps:
            for c in range(4):
                nc.sync.dma_start(out=xt[c][:, 0:1024], in_=ap_xt[c][:, 0:1024])
                nc.sync.dma_start(out=xt[c][:, 1024:], in_=ap_xt[c][:, 1024:])
                nc.sync.dma_start(out=wk_sb[c], in_=ap_wk[c])
                nc.sync.dma_start(out=wq_sb[c], in_=ap_wq[c])
                nc.sync.dma_start(out=wv_sb[c], in_=ap_wv[c])
            for icp in (0, 2):
                kps = [
                    proj_ps.tile([128, 512], F32, tag="proj", name="kps")
                    for _ in range(2)
                ]
                for c in range(4):
                    for k in range(2):
                        nc.tensor.matmul(
                            kps[k][:],
                            wk_sb[c][:],
                            xt[c][:, (icp + k) * 512 : (icp + k + 1) * 512],
                            start=(c == 0),
                            stop=(c == 3),
                        )
                for k in range(2):
                    nc.scalar.copy(
                        out=kT[:, (icp + k) * 512 : (icp + k + 1) * 512],
                        in_=kps[k][:],
                    )
            q0_ps = proj_ps.tile([128, 512], F32, tag="proj", name="q0ps")
            for c in range(4):
                nc.tensor.matmul(
                    q0_ps[:],
                    wq_sb[c][:],
                    xt[c][:, 0:512],
                    start=(c == 0),
                    stop=(c == 3),
                )
            nc.scalar.copy(out=qT[:, 0:512], in_=q0_ps[:])

        # ---- attention: i-quarter (512) outer; both heads share one scores
        # PSUM tile (h0 cols 0-511, h1 cols 512-1023) -> one exp call covers
        # both heads; sp double-buffered; av staggered one j-tile behind.
        # The out-projection for quarter q-1 rides inside quarter q's
        # ACT-bound window (PE and DVE have slack there); its PSUM tiles share
        # the 1-bank "mix" pool with the av accumulators and deferred q-proj.
        rd = [dpool.tile([n], F32, tag=f"rd{h}", name=f"rd{h}") for h in range(2)]
        n_iq = n // 512

        def outproj_quarter(iq, half, mix_pool, stage_pool):
            ts0 = iq * 4 + (2 if half else 0)
            for t in range(ts0, ts0 + 2):
                p0 = mix_pool.tile([128, 512], F32, tag="mix", name="p0")
                p1 = mix_pool.tile([128, 512], F32, tag="mix", name="p1")
                nc.tensor.matmul(
                    p0[:],
                    oT[0:64, t * 128 : (t + 1) * 128],
                    wo_sb[0:64, :],
                    start=True,
                    stop=True,
                )
                nc.tensor.matmul(
                    p1[:],
                    oT[64:128, t * 128 : (t + 1) * 128],
                    wo_sb[64:128, :],
                    start=True,
                    stop=True,
                )
                a1 = stage_pool.tile([128, 512], F32, tag="a1", name="a1")
                nc.vector.tensor_scalar_mul(a1[:], p1[:], rec_col[1][:, t : t + 1])
                outb = stage_pool.tile([128, 512], F32, tag="outb", name="outb")
                nc.vector.scalar_tensor_tensor(
                    out=outb[:],
                    in0=p0[:],
                    scalar=rec_col[0][:, t : t + 1],
                    in1=a1[:],
                    op0=ALU.mult,
                    op1=ALU.add,
                )
                eng = nc.sync if t % 2 == 0 else nc.scalar
                eng.dma_start(out=ap_out[t], in_=outb[:])

        with (
            tc.tile_pool(name="s_ps", bufs=2, space="PSUM") as s_ps_pool,
            tc.tile_pool(name="mix_ps", bufs=4, space="PSUM") as mix_ps,
            tc.tile_pool(name="exp_sb", bufs=7) as exp_pool,
            tc.tile_pool(name="small", bufs=4) as small,
            tc.tile_pool(name="ostage", bufs=4) as ostage,
        ):
            for iq in range(n_iq):
                i0 = iq * 512
                avs = [
                    mix_ps.tile([VA, 512], F32, tag="mix", name=f"av{h}")
                    for h in range(2)
                ]
                pending = []
                for jt in range(n_jt):
                    sp = s_ps_pool.tile([128, 1024], F32, tag="sp", name="sp")
                    for h in range(2):
                        hp = h * 64
                        nc.tensor.matmul(
                            sp[:, h * 512 : (h + 1) * 512],
                            kT[hp : hp + 64, jt * 128 : (jt + 1) * 128],
                            qT[hp : hp + 64, i0 : i0 + 512],
                            start=True,
                            stop=True,
                        )
                    et = exp_pool.tile([128, 1024], BF, tag="exp", name="et")
                    nc.scalar.activation(out=et[:], in_=sp[:], func=AF.Exp)
                    pending.append((et, jt))
                    if iq == 0 and jt == 3:
                        # vT projection + transposes ride inside the ACT-bound
                        # window; av for j-tiles 0..3 queues up behind them.
                        for icp in (0, 2):
                            vps = [
                                mix_ps.tile([128, 512], F32, tag="mix", name="vps")
                                for _ in range(2)
                            ]
                            for c in range(4):
                                for k in range(2):
                                    nc.tensor.matmul(
                                        vps[k][:],
                                        wv_sb[c][:],
                                        xt[c][:, (icp + k) * 512 : (icp + k + 1) * 512],
                                        start=(c == 0),
                                        stop=(c == 3),
                                    )
                            for k in range(2):
                                nc.vector.tensor_copy(
                                    out=vT[:, (icp + k) * 512 : (icp + k + 1) * 512],
                                    in_=vps[k][:],
                                )
                    if iq == 0 and jt == 5:
                        for tjt in range(n_jt - 1):
                            pt = mix_ps.tile([128, 128], F32, tag="mix", name="tr")
                            nc.tensor.transpose(
                                pt[:], vT[:, tjt * 128 : (tjt + 1) * 128], ident[:]
                            )
                            for h in range(2):
                                nc.vector.tensor_copy(
                                    out=v_aug[h][:, tjt * VA : tjt * VA + 64],
                                    in_=pt[:, h * 64 : (h + 1) * 64],
                                )
                    if len(pending) > 1 and (iq > 0 or jt >= 7):
                        pet, pjt = pending.pop(0)
                        for h in range(2):
                            nc.tensor.matmul(
                                avs[h][:],
                                v_aug[h][:, pjt * VA : (pjt + 1) * VA],
                                pet[:, h * 512 : (h + 1) * 512],
                                start=(pjt == 0),
                                stop=False,
                            )
                    if iq == 0 and jt in (10, 12, 14):
                        # deferred qT chunk, one per insertion point
                        ic = (jt - 10) // 2 + 1
                        qp = mix_ps.tile([128, 512], F32, tag="mix", name="qdef")
                        for c in range(4):
                            nc.tensor.matmul(
                                qp[:],
                                wq_sb[c][:],
                                xt[c][:, ic * 512 : (ic + 1) * 512],
                                start=(c == 0),
                                stop=(c == 3),
                            )
                        nc.vector.tensor_copy(
                            out=qT[:, ic * 512 : (ic + 1) * 512], in_=qp[:]
                        )
                    if jt in (4, 8) and iq >= 1:
                        # out-projection for the previous quarter rides here
                        # (2 tiles per insertion), round-trip long completed
                        outproj_quarter(iq - 1, jt == 8, mix_ps, ostage)
                while pending:
                    pet, pjt = pending.pop(0)
                    for h in range(2):
                        nc.tensor.matmul(
                            avs[h][:],
                            v_aug[h][:, pjt * VA : (pjt + 1) * VA],
                            pet[:, h * 512 : (h + 1) * 512],
                            start=(pjt == 0),
                            stop=(pjt == n_jt - 1),
                        )
                # epilogue for this i-quarter (overlaps next quarter's attention)
                for h in range(2):
                    den = small.tile([1, 512], F32, tag="den", name="den")
                    nc.vector.tensor_copy(out=den[:], in_=avs[h][64:65, :])
                    nc.gpsimd.dma_start(out=rd[h][i0 : i0 + 512], in_=den[:])
                    den_col = small.tile([128, 4], F32, tag="den_col", name="den_col")
                    nc.gpsimd.dma_start(
                        out=den_col[:],
                        in_=rd[h][i0 : i0 + 512].rearrange("(t p) -> p t", p=128),
                    )
                    nc.vector.reciprocal(
                        out=rec_col[h][:, iq * 4 : iq * 4 + 4], in_=den_col[:]
                    )
                for h in range(2):
                    hp = h * 64
                    nc.vector.tensor_copy(
                        out=oT[hp : hp + 64, i0 : i0 + 512], in_=avs[h][0:64, :]
                    )
            # final quarter's out-projection
            outproj_quarter(n_iq - 1, False, mix_ps, ostage)
            outproj_quarter(n_iq - 1, True, mix_ps, ostage)


def _build(n=NSEQ):
    if n in _CACHE:
        return _CACHE[n]
    nc = bacc.Bacc("TRN2", debug=False, num_devices=N_CORES)
    with tile.TileContext(nc) as tc:
        _emit(nc, tc, n)
    nc.compile()
    _CACHE[n] = nc
    return nc


def _prep_in_maps(x, wq, wkv, wo, m_k, m_v, n):
    x = np.asarray(x, np.float32)
    wq = np.asarray(wq, np.float32)
    wkv = np.asarray(wkv, np.float32)
    wo = np.asarray(wo, np.float32)
    m_k = np.asarray(m_k, np.float32)
    m_v = np.asarray(m_v, np.float32)

    wk = wkv[:, :INNER]
    wv = wkv[:, INNER:]
    # memory tokens: flat reshape (M, INNER) -> (H, M, DH), exactly as reference
    mk_heads = m_k.reshape(M * INNER).reshape(H, M, DH)  # * SQRT_DH * SCALE == 1.0
    mv_heads = m_v.reshape(M * INNER).reshape(H, M, DH) * SQRT_M

    in_maps = []
    for cid in range(N_CORES):
        b = cid // 4
        h0 = 2 * (cid % 4)
        sl = slice(h0 * DH, (h0 + 2) * DH)
        in_maps.append(
            {
                "xt": np.ascontiguousarray(x[b].T).reshape(4, 128, n),
                "wq_s": np.ascontiguousarray(wq[:, sl]).reshape(4, 128, 128),
                "wk_s": np.ascontiguousarray(wk[:, sl] * SCALE).reshape(4, 128, 128),
                "wv_s": np.ascontiguousarray(wv[:, sl]).reshape(4, 128, 128),
                "wo_s": np.ascontiguousarray(wo[sl, :]).astype(ml_dtypes.bfloat16),
                "mkT_s": np.ascontiguousarray(
                    np.concatenate([mk_heads[h0].T, mk_heads[h0 + 1].T], axis=0)
                ).astype(ml_dtypes.bfloat16),
                "mv_s": np.ascontiguousarray(
                    np.concatenate([mv_heads[h0], mv_heads[h0 + 1]], axis=1)
                ).astype(ml_dtypes.bfloat16),
            }
        )
    return in_maps


def _gather(results, bo, n):
    bo = np.asarray(bo, np.float32)
    out = np.zeros((B, n, DIM), np.float32)
    for cid in range(N_CORES):
        out[cid // 4] += results[cid]["out"].reshape(n, DIM)
    out += bo
    return out


def run(x, wq, wkv, wo, bo, m_k, m_v, trace=False, n=NSEQ):
    nc = _build(n)
    in_maps = _prep_in_maps(x, wq, wkv, wo, m_k, m_v, n)
    res = bass_utils.run_bass_kernel_spmd(
        nc, in_maps, core_ids=list(range(N_CORES)), trace=trace
    )
    return _gather(res.results, bo, n), res


def kernel(x, wq, wkv, wo, bo, m_k, m_v):
    out, _ = run(x, wq, wkv, wo, bo, m_k, m_v)
    return out



# revision 4
# speedup vs baseline: 1.1154x; 1.1154x over previous
"""CMAttention Trainium2 Bass kernel.

Reference computation (b=2, n=2048, dim=512, H=8 heads, dh=64, M=3 memory tokens):
    q = x @ wq;  k, v = split(x @ wkv);  per-head attention with 3 extra
    memory k/v tokens appended;  out = softmax(q k^T / 8) v;  y = out @ wo + bo.

Sharding: 16 (batch, head) pairs over 8 cores -> each core owns one batch and
two adjacent heads.  Per core everything is local; the out-projection is
row-sharded (per-head) and partial outputs are summed on the host (the
all-reduce of the sharding hint, done at gather time).

Device-side layout (per core, two heads "stacked" on partitions 0-63 / 64-127):
    xt   [4][128, 2048]   x[b]^T in bf16, contraction c on partitions
    qT   [128, 2048]      q^T = wq_s^T-chunks @ xt    (d_global on partitions)
    kT   [128, 2052]      k^T * 1/8 (scale folded into wk on host) ++ memory keys
    v    via PE transpose -> v_aug[h] [128, 17*65]: per j-tile [128, 64+1(ones)]
    scores^T s[j, i] = kT_h^T-slice.T @ qT_h  -> PSUM [128(j), 1024(i)]
      (the two heads' QK matmuls dual-issue on the PE's 64-row groups)
    exp on ScalarE PSUM->SBUF
    av:  out_h^T[65, i] += v_aug_jt.T @ exp_jt   (row 64 = softmax denominator)
    denominator row -> rec_col via tiny PE transposes (no DRAM round-trip)
    out-projection per head + per-partition reciprocal scaling, host sums partials.
"""

import sys

for _p in ("/opt/trn_rl_repo", "/root/.axon_site/_ro/trn_rl_repo"):
    if _p not in sys.path:
        sys.path.insert(0, _p)

import ml_dtypes
import numpy as np

import concourse.bacc as bacc
import concourse.mybir as mybir
import concourse.tile as tile
from concourse import bass_utils
from concourse.masks import make_identity

F32 = mybir.dt.float32
BF = mybir.dt.bfloat16
AF = mybir.ActivationFunctionType
ALU = mybir.AluOpType

H, DH, M = 8, 64, 3
DIM = 512
INNER = H * DH
NSEQ = 2048
B = 2
N_CORES = 8
SCALE = DH ** -0.5
SQRT_M = float(np.sqrt(M))

_CACHE = {}


def _emit(nc, tc, n):
    """Emit the per-core program. n = sequence length (queries)."""
    n_it = n // 128          # i-tiles of 128 queries
    n_jt = n // 128 + 1      # j-tiles: n/128 full + 1 memory tile (3 rows)
    VA = 65                  # v_aug cols per j-tile: 64 dims + ones column

    ap_xt = nc.dram_tensor("xt", [4, 128, n], BF, kind="ExternalInput").ap()
    ap_wq = nc.dram_tensor("wq_s", [4, 128, 128], BF, kind="ExternalInput").ap()
    ap_wk = nc.dram_tensor("wk_s", [4, 128, 128], BF, kind="ExternalInput").ap()
    ap_wv = nc.dram_tensor("wv_s", [4, 128, 128], BF, kind="ExternalInput").ap()
    ap_wo = nc.dram_tensor("wo_s", [128, DIM], BF, kind="ExternalInput").ap()
    ap_mkT = nc.dram_tensor("mkT_s", [128, M], BF, kind="ExternalInput").ap()
    ap_mv = nc.dram_tensor("mv_s", [M, 128], BF, kind="ExternalInput").ap()
    ap_out = nc.dram_tensor("out", [n_it, 128, DIM], F32, kind="ExternalOutput").ap()

    with (
        tc.tile_pool(name="persist", bufs=1) as per,
    ):
        xt = [per.tile([128, n], BF, tag=f"xt{c}", name=f"xt{c}") for c in range(4)]
        wq_sb = [per.tile([128, 128], BF, tag=f"wq{c}", name=f"wq{c}") for c in range(4)]
        wk_sb = [per.tile([128, 128], BF, tag=f"wk{c}", name=f"wk{c}") for c in range(4)]
        wv_sb = [per.tile([128, 128], BF, tag=f"wv{c}", name=f"wv{c}") for c in range(4)]
        wo_sb = per.tile([128, DIM], BF, tag="wo", name="wo")
        qT = per.tile([128, n], BF, tag="qT", name="qT")
        kT = per.tile([128, n + 128], BF, tag="kT", name="kT")
        vT = per.tile([128, n], BF, tag="vT", name="vT")
        v_aug = [per.tile([128, n_jt * VA], BF, tag=f"vaug{h}", name=f"vaug{h}") for h in range(2)]
        oT = per.tile([128, n], BF, tag="oT", name="oT")
        rec_col = [per.tile([128, n_it], F32, tag=f"rec{h}", name=f"rec{h}") for h in range(2)]
        ident = per.tile([128, 128], BF, tag="ident", name="ident")
        ones1 = per.tile([1, 1], F32, tag="ones1", name="ones1")

        # ---- input DMAs: spread issue across 4 engines so transfers start in
        # parallel; weights first on sync (small), xt column-halves on the
        # other engines in consumption order.
        nc.sync.dma_start(out=wk_sb[0], in_=ap_wk[0])
        nc.sync.dma_start(out=wk_sb[1], in_=ap_wk[1])
        nc.sync.dma_start(out=wk_sb[2], in_=ap_wk[2])
        nc.sync.dma_start(out=wk_sb[3], in_=ap_wk[3])
        dma_engs = [nc.scalar, nc.gpsimd]
        for c in range(4):
            eng = dma_engs[c % 2]
            eng.dma_start(out=xt[c][:, 0:1024], in_=ap_xt[c][:, 0:1024])
        for c in range(4):
            nc.sync.dma_start(out=wq_sb[c], in_=ap_wq[c])
        for c in range(4):
            eng = dma_engs[c % 2]
            eng.dma_start(out=xt[c][:, 1024:], in_=ap_xt[c][:, 1024:])
        for c in range(4):
            nc.sync.dma_start(out=wv_sb[c], in_=ap_wv[c])
        nc.sync.dma_start(out=wo_sb, in_=ap_wo)
        nc.vector.memset(kT[:, n : n + 128], 0.0)
        nc.sync.dma_start(out=kT[:, n : n + M], in_=ap_mkT)
        make_identity(nc, ident[:])
        nc.gpsimd.memset(ones1[:], 1.0)
        for h in range(2):
            nc.vector.memset(v_aug[h][:], 1.0)
            mb = (n_jt - 1) * VA
            nc.vector.memset(v_aug[h][:, mb : mb + VA], 0.0)
            nc.vector.memset(v_aug[h][0:M, mb + 64 : mb + VA], 1.0)
            nc.sync.dma_start(
                out=v_aug[h][0:M, mb : mb + 64],
                in_=ap_mv[:, h * 64 : (h + 1) * 64],
            )

        # ---- minimal pre-attention projections: kT (ic-major so early j-tiles
        # land first) and qT chunk 0. vT + transposes + remaining qT chunks are
        # deferred into the attention window.
        with tc.tile_pool(name="proj_ps", bufs=6, space="PSUM") as proj_ps:
            for icp in (0, 2):
                kps = [
                    proj_ps.tile([128, 512], F32, tag="proj", name="kps")
                    for _ in range(2)
                ]
                for c in range(4):
                    for k in range(2):
                        nc.tensor.matmul(
                            kps[k][:],
                            wk_sb[c][:],
                            xt[c][:, (icp + k) * 512 : (icp + k + 1) * 512],
                            start=(c == 0),
                            stop=(c == 3),
                        )
                for k in range(2):
                    nc.scalar.copy(
                        out=kT[:, (icp + k) * 512 : (icp + k + 1) * 512],
                        in_=kps[k][:],
                    )
            q0_ps = proj_ps.tile([128, 512], F32, tag="proj", name="q0ps")
            for c in range(4):
                nc.tensor.matmul(
                    q0_ps[:],
                    wq_sb[c][:],
                    xt[c][:, 0:512],
                    start=(c == 0),
                    stop=(c == 3),
                )
            nc.scalar.copy(out=qT[:, 0:512], in_=q0_ps[:])

        # ---- attention: i-quarter (512) outer; both heads share one scores
        # PSUM tile (h0 cols 0-511, h1 cols 512-1023) -> one exp call covers
        # both heads; sp double-buffered; av staggered one j-tile behind.
        # The out-projection for quarter q-1 rides inside quarter q's window;
        # its PSUM tiles share the "mix" pool with the av accumulators and
        # deferred q/v projections.
        n_iq = n // 512

        def outproj_quarter(iq, half, mix_pool, stage_pool):
            ts0 = iq * 4 + (2 if half else 0)
            for t in range(ts0, ts0 + 2):
                p0 = mix_pool.tile([128, 512], F32, tag="mix", name="p0")
                p1 = mix_pool.tile([128, 512], F32, tag="mix", name="p1")
                nc.tensor.matmul(
                    p0[:],
                    oT[0:64, t * 128 : (t + 1) * 128],
                    wo_sb[0:64, :],
                    start=True,
                    stop=True,
                )
                nc.tensor.matmul(
                    p1[:],
                    oT[64:128, t * 128 : (t + 1) * 128],
                    wo_sb[64:128, :],
                    start=True,
                    stop=True,
                )
                a1 = stage_pool.tile([128, 512], F32, tag="a1", name="a1")
                nc.vector.tensor_scalar_mul(a1[:], p1[:], rec_col[1][:, t : t + 1])
                outb = stage_pool.tile([128, 512], F32, tag="outb", name="outb")
                nc.vector.scalar_tensor_tensor(
                    out=outb[:],
                    in0=p0[:],
                    scalar=rec_col[0][:, t : t + 1],
                    in1=a1[:],
                    op0=ALU.mult,
                    op1=ALU.add,
                )
                eng = nc.sync if t % 2 == 0 else nc.gpsimd
                eng.dma_start(out=ap_out[t], in_=outb[:])

        with (
            tc.tile_pool(name="s_ps", bufs=2, space="PSUM") as s_ps_pool,
            tc.tile_pool(name="mix_ps", bufs=4, space="PSUM") as mix_ps,
            tc.tile_pool(name="exp_sb", bufs=7) as exp_pool,
            tc.tile_pool(name="small", bufs=4) as small,
            tc.tile_pool(name="ostage", bufs=4) as ostage,
        ):
            for iq in range(n_iq):
                i0 = iq * 512
                avs = [
                    mix_ps.tile([VA, 512], F32, tag="mix", name=f"av{h}")
                    for h in range(2)
                ]
                pending = []
                for jt in range(n_jt):
                    sp = s_ps_pool.tile([128, 1024], F32, tag="sp", name="sp")
                    for h in range(2):
                        hp = h * 64
                        nc.tensor.matmul(
                            sp[:, h * 512 : (h + 1) * 512],
                            kT[hp : hp + 64, jt * 128 : (jt + 1) * 128],
                            qT[hp : hp + 64, i0 : i0 + 512],
                            start=True,
                            stop=True,
                        )
                    et = exp_pool.tile([128, 1024], BF, tag="exp", name="et")
                    nc.scalar.activation(out=et[:], in_=sp[:], func=AF.Exp)
                    pending.append((et, jt))
                    if iq == 0 and jt == 3:
                        # vT projection rides inside the early window; av for
                        # j-tiles 0..3 queues up behind it.
                        for icp in (0, 2):
                            vps = [
                                mix_ps.tile([128, 512], F32, tag="mix", name="vps")
                                for _ in range(2)
                            ]
                            for c in range(4):
                                for k in range(2):
                                    nc.tensor.matmul(
                                        vps[k][:],
                                        wv_sb[c][:],
                                        xt[c][:, (icp + k) * 512 : (icp + k + 1) * 512],
                                        start=(c == 0),
                                        stop=(c == 3),
                                    )
                            for k in range(2):
                                nc.vector.tensor_copy(
                                    out=vT[:, (icp + k) * 512 : (icp + k + 1) * 512],
                                    in_=vps[k][:],
                                )
                    if iq == 0 and jt == 5:
                        for tjt in range(n_jt - 1):
                            pt = mix_ps.tile([128, 128], BF, tag="mix", name="tr")
                            nc.tensor.transpose(
                                pt[:], vT[:, tjt * 128 : (tjt + 1) * 128], ident[:]
                            )
                            for h in range(2):
                                nc.vector.tensor_copy(
                                    out=v_aug[h][:, tjt * VA : tjt * VA + 64],
                                    in_=pt[:, h * 64 : (h + 1) * 64],
                                )
                    if len(pending) > 1 and (iq > 0 or jt >= 7):
                        pet, pjt = pending.pop(0)
                        for h in range(2):
                            nc.tensor.matmul(
                                avs[h][:],
                                v_aug[h][:, pjt * VA : (pjt + 1) * VA],
                                pet[:, h * 512 : (h + 1) * 512],
                                start=(pjt == 0),
                                stop=False,
                            )
                    if iq == 0 and jt in (10, 12, 14):
                        # deferred qT chunk, one per insertion point
                        ic = (jt - 10) // 2 + 1
                        qp = mix_ps.tile([128, 512], F32, tag="mix", name="qdef")
                        for c in range(4):
                            nc.tensor.matmul(
                                qp[:],
                                wq_sb[c][:],
                                xt[c][:, ic * 512 : (ic + 1) * 512],
                                start=(c == 0),
                                stop=(c == 3),
                            )
                        nc.vector.tensor_copy(
                            out=qT[:, ic * 512 : (ic + 1) * 512], in_=qp[:]
                        )
                    if jt in (4, 8) and iq >= 1:
                        # out-projection for the previous quarter rides here
                        # (2 tiles per insertion), round-trip long completed
                        outproj_quarter(iq - 1, jt == 8, mix_ps, ostage)
                while pending:
                    pet, pjt = pending.pop(0)
                    for h in range(2):
                        nc.tensor.matmul(
                            avs[h][:],
                            v_aug[h][:, pjt * VA : (pjt + 1) * VA],
                            pet[:, h * 512 : (h + 1) * 512],
                            start=(pjt == 0),
                            stop=(pjt == n_jt - 1),
                        )
                # epilogue for this i-quarter (overlaps next quarter's attention).
                # denominator row -> column layout via 4 tiny PE transposes
                # (1-col streams), no DRAM round trip.
                for h in range(2):
                    den = small.tile([1, 512], F32, tag="den", name="den")
                    nc.vector.tensor_copy(out=den[:], in_=avs[h][64:65, :])
                    dc = mix_ps.tile([128, 4], F32, tag="mix", name="dc")
                    for t in range(4):
                        nc.tensor.transpose(
                            dc[:, t : t + 1],
                            den[0:1, t * 128 : (t + 1) * 128],
                            ones1[:],
                        )
                    nc.vector.reciprocal(
                        out=rec_col[h][:, iq * 4 : iq * 4 + 4], in_=dc[:]
                    )
                for h in range(2):
                    hp = h * 64
                    nc.vector.tensor_copy(
                        out=oT[hp : hp + 64, i0 : i0 + 512], in_=avs[h][0:64, :]
                    )
            # final quarter's out-projection
            outproj_quarter(n_iq - 1, False, mix_ps, ostage)
            outproj_quarter(n_iq - 1, True, mix_ps, ostage)


def _build(n=NSEQ):
    if n in _CACHE:
        return _CACHE[n]
    nc = bacc.Bacc("TRN2", debug=False, num_devices=N_CORES)
    with tile.TileContext(nc) as tc:
        _emit(nc, tc, n)
    nc.compile()
    _CACHE[n] = nc
    return nc


def _prep_in_maps(x, wq, wkv, wo, m_k, m_v, n):
    x = np.asarray(x, np.float32)
    wq = np.asarray(wq, np.float32)
    wkv = np.asarray(wkv, np.float32)
    wo = np.asarray(wo, np.float32)
    m_k = np.asarray(m_k, np.float32)
    m_v = np.asarray(m_v, np.float32)

    wk = wkv[:, :INNER]
    wv = wkv[:, INNER:]
    # memory tokens: flat reshape (M, INNER) -> (H, M, DH), exactly as reference
    mk_heads = m_k.reshape(M * INNER).reshape(H, M, DH)  # * SQRT_DH * SCALE == 1.0
    mv_heads = m_v.reshape(M * INNER).reshape(H, M, DH) * SQRT_M

    in_maps = []
    for cid in range(N_CORES):
        b = cid // 4
        h0 = 2 * (cid % 4)
        sl = slice(h0 * DH, (h0 + 2) * DH)
        in_maps.append(
            {
                "xt": np.ascontiguousarray(x[b].T)
                .reshape(4, 128, n)
                .astype(ml_dtypes.bfloat16),
                "wq_s": np.ascontiguousarray(wq[:, sl])
                .reshape(4, 128, 128)
                .astype(ml_dtypes.bfloat16),
                "wk_s": np.ascontiguousarray(wk[:, sl] * SCALE)
                .reshape(4, 128, 128)
                .astype(ml_dtypes.bfloat16),
                "wv_s": np.ascontiguousarray(wv[:, sl])
                .reshape(4, 128, 128)
                .astype(ml_dtypes.bfloat16),
                "wo_s": np.ascontiguousarray(wo[sl, :]).astype(ml_dtypes.bfloat16),
                "mkT_s": np.ascontiguousarray(
                    np.concatenate([mk_heads[h0].T, mk_heads[h0 + 1].T], axis=0)
                ).astype(ml_dtypes.bfloat16),
                "mv_s": np.ascontiguousarray(
                    np.concatenate([mv_heads[h0], mv_heads[h0 + 1]], axis=1)
                ).astype(ml_dtypes.bfloat16),
            }
        )
    return in_maps


def _gather(results, bo, n):
    bo = np.asarray(bo, np.float32)
    out = np.zeros((B, n, DIM), np.float32)
    for cid in range(N_CORES):
        out[cid // 4] += results[cid]["out"].reshape(n, DIM)
    out += bo
    return out


def run(x, wq, wkv, wo, bo, m_k, m_v, trace=False, n=NSEQ):
    nc = _build(n)
    in_maps = _prep_in_maps(x, wq, wkv, wo, m_k, m_v, n)
    res = bass_utils.run_bass_kernel_spmd(
        nc, in_maps, core_ids=list(range(N_CORES)), trace=trace
    )
    return _gather(res.results, bo, n), res


def kernel(x, wq, wkv, wo, bo, m_k, m_v):
    out, _ = run(x, wq, wkv, wo, bo, m_k, m_v)
    return out


# revision 11
# speedup vs baseline: 1.1972x; 1.0734x over previous
"""CMAttention Trainium2 Bass kernel.

Reference computation (b=2, n=2048, dim=512, H=8 heads, dh=64, M=3 memory tokens):
    q = x @ wq;  k, v = split(x @ wkv);  per-head attention with 3 extra
    memory k/v tokens appended;  out = softmax(q k^T / 8) v;  y = out @ wo + bo.

Sharding: 16 (batch, head) pairs over 8 cores -> each core owns one batch and
two adjacent heads.  Per core everything is local; the out-projection is
row-sharded (per-head) and partial outputs are summed on the host (the
all-reduce of the sharding hint, done at gather time).

Device-side layout (per core, two heads "stacked" on partitions 0-63 / 64-127):
    xt   [4][128, 2048]   x[b]^T in bf16, contraction c on partitions
    qT   [128, 2048]      q^T = wq_s^T-chunks @ xt    (d_global on partitions)
    kT   [128, 2052]      k^T * 1/8 (scale folded into wk on host) ++ memory keys
    v    via PE transpose -> v_aug[h] [128, 17*65]: per j-tile [128, 64+1(ones)]
    scores^T s[j, i] = kT_h^T-slice.T @ qT_h  -> PSUM [128(j), 1024(i)]
      (the two heads' QK matmuls dual-issue on the PE's 64-row groups)
    exp on ScalarE PSUM->SBUF
    av:  out_h^T[65, i] += v_aug_jt.T @ exp_jt   (row 64 = softmax denominator)
    denominator row -> rec_col via tiny PE transposes (no DRAM round-trip)
    out-projection per head + per-partition reciprocal scaling, host sums partials.
"""

import sys

for _p in ("/opt/trn_rl_repo", "/root/.axon_site/_ro/trn_rl_repo"):
    if _p not in sys.path:
        sys.path.insert(0, _p)

import ml_dtypes
import numpy as np

import concourse.bacc as bacc
import concourse.mybir as mybir
import concourse.tile as tile
from concourse import bass_utils
from concourse.masks import make_identity

F32 = mybir.dt.float32
BF = mybir.dt.bfloat16
AF = mybir.ActivationFunctionType
ALU = mybir.AluOpType

H, DH, M = 8, 64, 3
DIM = 512
INNER = H * DH
NSEQ = 2048
B = 2
N_CORES = 8
SCALE = DH ** -0.5
SQRT_M = float(np.sqrt(M))

_CACHE = {}


def _emit(nc, tc, n):
    """Emit the per-core program. n = sequence length (queries)."""
    n_it = n // 128          # i-tiles of 128 queries
    n_jt = n // 128 + 1      # j-tiles: n/128 full + 1 memory tile (3 rows)
    VA = 65                  # v_aug cols per j-tile: 64 dims + ones column

    ap_xt = nc.dram_tensor("xt", [4, 128, n], BF, kind="ExternalInput").ap()
    # weights stored chunk-major along columns: [128, 4*128], col block c =
    # contraction chunk c -> one contiguous 1KB-row DMA instead of 4 small ones
    ap_wq = nc.dram_tensor("wq_s", [128, 512], BF, kind="ExternalInput").ap()
    ap_wk = nc.dram_tensor("wk_s", [128, 512], BF, kind="ExternalInput").ap()
    ap_wv = nc.dram_tensor("wv_s", [128, 512], BF, kind="ExternalInput").ap()
    ap_wo = nc.dram_tensor("wo_s", [128, DIM], BF, kind="ExternalInput").ap()
    ap_mkT = nc.dram_tensor("mkT_s", [128, M], BF, kind="ExternalInput").ap()
    ap_mv = nc.dram_tensor("mv_s", [M, 128], BF, kind="ExternalInput").ap()
    ap_out = nc.dram_tensor("out", [n_it, 128, DIM], BF, kind="ExternalOutput").ap()

    with (
        tc.tile_pool(name="persist", bufs=1) as per,
    ):
        xt = [per.tile([128, n], BF, tag=f"xt{c}", name=f"xt{c}") for c in range(4)]
        wq_all = per.tile([128, 512], BF, tag="wq", name="wq")
        wk_all = per.tile([128, 512], BF, tag="wk", name="wk")
        wv_all = per.tile([128, 512], BF, tag="wv", name="wv")
        wq_sb = [wq_all[:, c * 128 : (c + 1) * 128] for c in range(4)]
        wk_sb = [wk_all[:, c * 128 : (c + 1) * 128] for c in range(4)]
        wv_sb = [wv_all[:, c * 128 : (c + 1) * 128] for c in range(4)]
        wo_sb = per.tile([128, DIM], BF, tag="wo", name="wo")
        qT = per.tile([128, n], BF, tag="qT", name="qT")
        kT = per.tile([128, n + 128], BF, tag="kT", name="kT")
        vT = per.tile([128, n], BF, tag="vT", name="vT")
        v_aug = [per.tile([128, n_jt * VA], BF, tag=f"vaug{h}", name=f"vaug{h}") for h in range(2)]
        oT = per.tile([128, n], BF, tag="oT", name="oT")
        rec_col = [per.tile([128, n_it], F32, tag=f"rec{h}", name=f"rec{h}") for h in range(2)]
        ident = per.tile([128, 128], BF, tag="ident", name="ident")
        ones1 = per.tile([1, 1], F32, tag="ones1", name="ones1")

        # ---- input DMAs: 3 hardware queues (sync/scalar/gpsimd), assigned in
        # consumption order so each ~110GB/s queue feeds the projection loop
        # just in time.  a = cols [0:1024] (icp=0), b = cols [1024:2048].
        def xa(c):
            return (xt[c][:, 0:1024], ap_xt[c][:, 0:1024])

        def xb(c):
            return (xt[c][:, 1024:], ap_xt[c][:, 1024:])

        for out_ap, in_ap in (xa(0), xa(2), xb(1)):
            nc.scalar.dma_start(out=out_ap, in_=in_ap)
        for out_ap, in_ap in (xa(1), xa(3), xb(0)):
            nc.gpsimd.dma_start(out=out_ap, in_=in_ap)
        nc.sync.dma_start(out=wk_all, in_=ap_wk)
        nc.sync.dma_start(out=wq_all, in_=ap_wq)
        for out_ap, in_ap in (xb(3), xb(2)):
            nc.sync.dma_start(out=out_ap, in_=in_ap)
        nc.sync.dma_start(out=wv_all, in_=ap_wv)
        nc.sync.dma_start(out=wo_sb, in_=ap_wo)
        nc.vector.memset(kT[:, n : n + 128], 0.0)
        nc.sync.dma_start(out=kT[:, n : n + M], in_=ap_mkT)
        make_identity(nc, ident[:])
        nc.gpsimd.memset(ones1[:], 1.0)
        for h in range(2):
            nc.vector.memset(v_aug[h][:], 1.0)
            mb = (n_jt - 1) * VA
            nc.vector.memset(v_aug[h][:, mb : mb + VA], 0.0)
            nc.vector.memset(v_aug[h][0:M, mb + 64 : mb + VA], 1.0)
            nc.sync.dma_start(
                out=v_aug[h][0:M, mb : mb + 64],
                in_=ap_mv[:, h * 64 : (h + 1) * 64],
            )

        # ---- minimal pre-attention projections: kT (ic-major so early j-tiles
        # land first) and qT chunk 0. vT + transposes + remaining qT chunks are
        # deferred into the attention window.
        with tc.tile_pool(name="proj_ps", bufs=6, space="PSUM") as proj_ps:
            for icp in (0, 2):
                kps = [
                    proj_ps.tile([128, 512], F32, tag="proj", name="kps")
                    for _ in range(2)
                ]
                for c in range(4):
                    for k in range(2):
                        nc.tensor.matmul(
                            kps[k][:],
                            wk_sb[c][:],
                            xt[c][:, (icp + k) * 512 : (icp + k + 1) * 512],
                            start=(c == 0),
                            stop=(c == 3),
                        )
                for k in range(2):
                    nc.scalar.copy(
                        out=kT[:, (icp + k) * 512 : (icp + k + 1) * 512],
                        in_=kps[k][:],
                    )
            q0_ps = proj_ps.tile([128, 512], F32, tag="proj", name="q0ps")
            for c in range(4):
                nc.tensor.matmul(
                    q0_ps[:],
                    wq_sb[c][:],
                    xt[c][:, 0:512],
                    start=(c == 0),
                    stop=(c == 3),
                )
            nc.scalar.copy(out=qT[:, 0:512], in_=q0_ps[:])

        # ---- attention: i-quarter (512) outer; both heads share one scores
        # PSUM tile (h0 cols 0-511, h1 cols 512-1023) -> one exp call covers
        # both heads; sp double-buffered; av staggered one j-tile behind.
        # The out-projection for quarter q-1 rides inside quarter q's window;
        # its PSUM tiles share the "mix" pool with the av accumulators and
        # deferred q/v projections.
        n_iq = n // 512

        def outproj_quarter(iq, half, mix_pool, stage_pool):
            ts0 = iq * 4 + (2 if half else 0)
            for t in range(ts0, ts0 + 2):
                p0 = mix_pool.tile([128, 512], F32, tag="mix", name="p0")
                p1 = mix_pool.tile([128, 512], F32, tag="mix", name="p1")
                nc.tensor.matmul(
                    p0[:],
                    oT[0:64, t * 128 : (t + 1) * 128],
                    wo_sb[0:64, :],
                    start=True,
                    stop=True,
                )
                nc.tensor.matmul(
                    p1[:],
                    oT[64:128, t * 128 : (t + 1) * 128],
                    wo_sb[64:128, :],
                    start=True,
                    stop=True,
                )
                a1 = stage_pool.tile([128, 512], F32, tag="a1", name="a1")
                nc.vector.tensor_scalar_mul(a1[:], p1[:], rec_col[1][:, t : t + 1])
                outb = stage_pool.tile([128, 512], BF, tag="outb", name="outb")
                nc.vector.scalar_tensor_tensor(
                    out=outb[:],
                    in0=p0[:],
                    scalar=rec_col[0][:, t : t + 1],
                    in1=a1[:],
                    op0=ALU.mult,
                    op1=ALU.add,
                )
                eng = nc.sync if t % 2 == 0 else nc.gpsimd
                eng.dma_start(out=ap_out[t], in_=outb[:])

        with (
            tc.tile_pool(name="s_ps", bufs=2, space="PSUM") as s_ps_pool,
            tc.tile_pool(name="mix_ps", bufs=4, space="PSUM") as mix_ps,
            tc.tile_pool(name="exp_sb", bufs=7) as exp_pool,
            tc.tile_pool(name="small", bufs=4) as small,
            tc.tile_pool(name="ostage", bufs=4) as ostage,
        ):
            for iq in range(n_iq):
                i0 = iq * 512
                avs = [
                    mix_ps.tile([VA, 512], F32, tag="mix", name=f"av{h}")
                    for h in range(2)
                ]
                pending = []
                for jt in range(n_jt):
                    sp = s_ps_pool.tile([128, 1024], F32, tag="sp", name="sp")
                    for h in range(2):
                        hp = h * 64
                        nc.tensor.matmul(
                            sp[:, h * 512 : (h + 1) * 512],
                            kT[hp : hp + 64, jt * 128 : (jt + 1) * 128],
                            qT[hp : hp + 64, i0 : i0 + 512],
                            start=True,
                            stop=True,
                        )
                    et = exp_pool.tile([128, 1024], BF, tag="exp", name="et")
                    nc.scalar.activation(out=et[:], in_=sp[:], func=AF.Exp)
                    pending.append((et, jt))
                    if iq == 0 and jt == 3:
                        # vT projection rides inside the early window; av for
                        # j-tiles 0..3 queues up behind it.
                        for icp in (0, 2):
                            vps = [
                                mix_ps.tile([128, 512], F32, tag="mix", name="vps")
                                for _ in range(2)
                            ]
                            for c in range(4):
                                for k in range(2):
                                    nc.tensor.matmul(
                                        vps[k][:],
                                        wv_sb[c][:],
                                        xt[c][:, (icp + k) * 512 : (icp + k + 1) * 512],
                                        start=(c == 0),
                                        stop=(c == 3),
                                    )
                            for k in range(2):
                                nc.vector.tensor_copy(
                                    out=vT[:, (icp + k) * 512 : (icp + k + 1) * 512],
                                    in_=vps[k][:],
                                )
                    if iq == 0 and jt == 5:
                        for tjt in range(n_jt - 1):
                            pt = mix_ps.tile([128, 128], BF, tag="mix", name="tr")
                            nc.tensor.transpose(
                                pt[:], vT[:, tjt * 128 : (tjt + 1) * 128], ident[:]
                            )
                            for h in range(2):
                                nc.vector.tensor_copy(
                                    out=v_aug[h][:, tjt * VA : tjt * VA + 64],
                                    in_=pt[:, h * 64 : (h + 1) * 64],
                                )
                    if len(pending) > 2 and (iq > 0 or jt >= 7):
                        pet, pjt = pending.pop(0)
                        for h in range(2):
                            nc.tensor.matmul(
                                avs[h][:],
                                v_aug[h][:, pjt * VA : (pjt + 1) * VA],
                                pet[:, h * 512 : (h + 1) * 512],
                                start=(pjt == 0),
                                stop=False,
                            )
                    if iq == 0 and jt in (10, 12, 14):
                        # deferred qT chunk, one per insertion point
                        ic = (jt - 10) // 2 + 1
                        qp = mix_ps.tile([128, 512], F32, tag="mix", name="qdef")
                        for c in range(4):
                            nc.tensor.matmul(
                                qp[:],
                                wq_sb[c][:],
                                xt[c][:, ic * 512 : (ic + 1) * 512],
                                start=(c == 0),
                                stop=(c == 3),
                            )
                        nc.vector.tensor_copy(
                            out=qT[:, ic * 512 : (ic + 1) * 512], in_=qp[:]
                        )
                    if jt in (4, 8) and iq >= 1:
                        # out-projection for the previous quarter rides here
                        # (2 tiles per insertion), round-trip long completed
                        outproj_quarter(iq - 1, jt == 8, mix_ps, ostage)
                while pending:
                    pet, pjt = pending.pop(0)
                    for h in range(2):
                        nc.tensor.matmul(
                            avs[h][:],
                            v_aug[h][:, pjt * VA : (pjt + 1) * VA],
                            pet[:, h * 512 : (h + 1) * 512],
                            start=(pjt == 0),
                            stop=(pjt == n_jt - 1),
                        )
                # epilogue for this i-quarter (overlaps next quarter's attention).
                # denominator row -> column layout via 4 tiny PE transposes
                # (1-col streams), no DRAM round trip.
                for h in range(2):
                    den = small.tile([1, 512], F32, tag="den", name="den")
                    nc.vector.tensor_copy(out=den[:], in_=avs[h][64:65, :])
                    dc = mix_ps.tile([128, 4], F32, tag="mix", name="dc")
                    for t in range(4):
                        nc.tensor.transpose(
                            dc[:, t : t + 1],
                            den[0:1, t * 128 : (t + 1) * 128],
                            ones1[:],
                        )
                    nc.vector.reciprocal(
                        out=rec_col[h][:, iq * 4 : iq * 4 + 4], in_=dc[:]
                    )
                for h in range(2):
                    hp = h * 64
                    nc.vector.tensor_copy(
                        out=oT[hp : hp + 64, i0 : i0 + 512], in_=avs[h][0:64, :]
                    )
            # final quarter's out-projection
            outproj_quarter(n_iq - 1, False, mix_ps, ostage)
            outproj_quarter(n_iq - 1, True, mix_ps, ostage)


def _build(n=NSEQ):
    if n in _CACHE:
        return _CACHE[n]
    nc = bacc.Bacc("TRN2", debug=False, num_devices=N_CORES)
    with tile.TileContext(nc) as tc:
        _emit(nc, tc, n)
    nc.compile()
    _CACHE[n] = nc
    return nc


def _prep_in_maps(x, wq, wkv, wo, m_k, m_v, n):
    x = np.asarray(x, np.float32)
    wq = np.asarray(wq, np.float32)
    wkv = np.asarray(wkv, np.float32)
    wo = np.asarray(wo, np.float32)
    m_k = np.asarray(m_k, np.float32)
    m_v = np.asarray(m_v, np.float32)

    wk = wkv[:, :INNER]
    wv = wkv[:, INNER:]
    # memory tokens: flat reshape (M, INNER) -> (H, M, DH), exactly as reference
    mk_heads = m_k.reshape(M * INNER).reshape(H, M, DH)  # * SQRT_DH * SCALE == 1.0
    mv_heads = m_v.reshape(M * INNER).reshape(H, M, DH) * SQRT_M

    in_maps = []
    for cid in range(N_CORES):
        b = cid // 4
        h0 = 2 * (cid % 4)
        sl = slice(h0 * DH, (h0 + 2) * DH)
        in_maps.append(
            {
                "xt": np.ascontiguousarray(x[b].T)
                .reshape(4, 128, n)
                .astype(ml_dtypes.bfloat16),
                "wq_s": np.ascontiguousarray(
                    wq[:, sl].reshape(4, 128, 128).transpose(1, 0, 2).reshape(128, 512)
                ).astype(ml_dtypes.bfloat16),
                "wk_s": np.ascontiguousarray(
                    (wk[:, sl] * SCALE)
                    .reshape(4, 128, 128)
                    .transpose(1, 0, 2)
                    .reshape(128, 512)
                ).astype(ml_dtypes.bfloat16),
                "wv_s": np.ascontiguousarray(
                    wv[:, sl].reshape(4, 128, 128).transpose(1, 0, 2).reshape(128, 512)
                ).astype(ml_dtypes.bfloat16),
                "wo_s": np.ascontiguousarray(wo[sl, :]).astype(ml_dtypes.bfloat16),
                "mkT_s": np.ascontiguousarray(
                    np.concatenate([mk_heads[h0].T, mk_heads[h0 + 1].T], axis=0)
                ).astype(ml_dtypes.bfloat16),
                "mv_s": np.ascontiguousarray(
                    np.concatenate([mv_heads[h0], mv_heads[h0 + 1]], axis=1)
                ).astype(ml_dtypes.bfloat16),
            }
        )
    return in_maps


def _gather(results, bo, n):
    bo = np.asarray(bo, np.float32)
    out = np.zeros((B, n, DIM), np.float32)
    for cid in range(N_CORES):
        out[cid // 4] += results[cid]["out"].reshape(n, DIM).astype(np.float32)
    out += bo
    return out


def run(x, wq, wkv, wo, bo, m_k, m_v, trace=False, n=NSEQ):
    nc = _build(n)
    in_maps = _prep_in_maps(x, wq, wkv, wo, m_k, m_v, n)
    res = bass_utils.run_bass_kernel_spmd(
        nc, in_maps, core_ids=list(range(N_CORES)), trace=trace
    )
    return _gather(res.results, bo, n), res


def kernel(x, wq, wkv, wo, bo, m_k, m_v):
    out, _ = run(x, wq, wkv, wo, bo, m_k, m_v)
    return out
